# revision 34
# baseline (speedup 1.0000x reference)
"""GatedAttention Trainium2 kernel.

Math (per batch b):
  Qw = x @ Wq + bq            (N, A)
  Kw = x @ Wk + bk            (N, A)
  g  = sigmoid(Qw @ Wv + bv)  (N,)
  S  = Qw @ Kw^T, diag -> -inf
  P  = softmax(S, axis=0)     (column softmax)
  out = (1-g)[:,None] * P + g[:,None] * I

Sharding: 8 cores = 4 batches x 2 column-halves of the score matrix.
Column softmax is independent per column, so no cross-core reduction.

Device layout: scores computed transposed, sT[j, i] tiles (j on partitions)
so the softmax reduction over i is a free-axis reduction. The i axis is
host-permuted so each core's diagonal block sits at i in [0, 2048) —
this keeps the program identical across cores (pure SPMD).

Pipeline per core:
  Projections (per 512-wide i-block, processed 4..7 then 0..3 in pairs):
  x arrives as [128, 1024] pair-tiles via SWDGE cast-DMA straight into
  fp32r (rounds in flight); weights are host-packed [128, 8, A] and land
  in two packed cast-DMAs each. The first block's Q accumulation is
  split into h0-3/h4-7 brackets fed by the sync and gpsimd DMA queues in
  parallel so the PE starts ~5us earlier and HAM never re-throttles.
  The gate (z = Qw @ Wv, g = 0.5+0.5*tanh(z/2); tanh shares the exp ACT
  table set) is software-pipelined one block behind, its PE ops
  interleaved between the next block's Q-groups so the ACT/DVE chain
  never stalls the strict-FIFO PE queue; the final gate lands inside
  the main loop's first tile. Diagonal planes dzp = ident * g are
  precomputed.
  Main loop per 128-column tile t: score matmuls -> PSUM, diag -1e30,
  exp (bf16 out, fp32 row-sum accum), 1/sum via fast reciprocal, bf16
  tensor_scalar (x 1/denom) + tensor_tensor (x (1-g_i)) passes, diag
  adds dzp, output streams out per 1024-wide chunk (bf16, host casts
  back to fp32).
"""
import numpy as np

import concourse.bacc as bacc
import concourse.mybir as mybir
import concourse.tile as tile
from concourse.bass_utils import run_bass_kernel_spmd

FP32 = mybir.dt.float32
FP32R = mybir.dt.float32r
BF16 = mybir.dt.bfloat16
AF = mybir.ActivationFunctionType
ALU = mybir.AluOpType

B, N, H, A = 4, 4096, 1024, 512
NSH = N // 2          # per-core column shard
NEG = -1.0e30

_CACHE = {}


def _build():
    nc = bacc.Bacc("TRN2", target_bir_lowering=False, debug=False, num_devices=8)
    xq = nc.dram_tensor("xq", [128, 8, N], FP32, kind="ExternalInput").ap()
    wq = nc.dram_tensor("wq", [H, A], FP32, kind="ExternalInput").ap()
    wk = nc.dram_tensor("wk", [H, A], FP32, kind="ExternalInput").ap()
    misc = nc.dram_tensor("misc", [128, 18], FP32, kind="ExternalInput").ap()
    out = nc.dram_tensor("out", [NSH, N], BF16, kind="ExternalOutput").ap()

    with tile.TileContext(nc) as tc:
        with (
            tc.tile_pool(name="const", bufs=1) as cpool,
            tc.tile_pool(name="proj_out", bufs=1) as qkpool,
            tc.tile_pool(name="gate", bufs=1) as gpool,
            tc.tile_pool(name="zrowps", bufs=1, space="PSUM") as zpool,
            tc.tile_pool(name="bcps", bufs=1, space="PSUM") as bps,
            tc.tile_pool(name="rowtmp", bufs=1) as rtmp,
        ):
            misc_sb = cpool.tile([128, 18], FP32, tag="misc", name="misc")
            nc.sync.dma_start(misc_sb[:], misc)

            # ---- PE warmup: dependency-free dummy matmuls issued while the
            # first operand DMAs are in flight, so the HAM activity window
            # opens before real work arrives (otherwise the first ~60 real
            # matmuls run at the cold 1.2 GHz rate). fp32 x fp32 on a
            # memset row — no fp32r rounding constraints involved.
            ones_f = cpool.tile([1, 128], FP32, tag="onesf", name="onesf")
            nc.vector.memset(ones_f[:], 1.0)
            warm = bps.tile([128, 512], FP32, tag="pb", name="warm")
            for _ in range(12):
                nc.tensor.matmul(warm[:, 0:128], ones_f[0:1, :], ones_f[0:1, :],
                                 start=True, stop=True)

            # ---- persistent projection outputs (fp32r) + gate planes ----
            qwt = [qkpool.tile([128, N], FP32R, tag=f"qwt{a}", name=f"qwt{a}") for a in range(4)]
            kwt = [qkpool.tile([128, NSH], FP32R, tag=f"kwt{a}", name=f"kwt{a}") for a in range(4)]
            g1m_bc = gpool.tile([128, N], BF16, tag="g1mbc", name="g1mbc")
            dzp = gpool.tile([128, NSH], BF16, tag="dzp", name="dzp")

            # gate stages, software-pipelined one i-block behind the
            # projections (stage A: z matmuls + tanh; B: (1-g) row +
            # broadcast; C: g row + diag plane)
            def gate_a(ib):
                sl = slice(ib * 512, (ib + 1) * 512)
                pzc = zpool.tile([2, 512], FP32, tag="zr", name="zr")
                for a in range(4):
                    nc.tensor.matmul(pzc[:], misc_r[:, 8 + 2 * a:10 + 2 * a],
                                     qwt[a][:, sl], start=(a == 0), stop=(a == 3))
                y_row = rtmp.tile([1, 512], FP32, tag="y", name="y")
                nc.scalar.activation(y_row[:], pzc[0:1, :], AF.Tanh,
                                     bias=misc_sb[0:1, 16:17], scale=0.5)
                return y_row

            def gate_b(ib, y_row):
                sl = slice(ib * 512, (ib + 1) * 512)
                g1m_row = rtmp.tile([1, 512], FP32R, tag="g1m", name="g1m")
                nc.vector.tensor_scalar(g1m_row[:], y_row[:],
                                        -0.5, 0.5, op0=ALU.mult, op1=ALU.add)
                pb = bps.tile([128, 512], FP32, tag="pb", name="pb")
                nc.tensor.matmul(pb[:], ones_r[0:1, :], g1m_row[:],
                                 start=True, stop=True)
                nc.vector.tensor_copy(g1m_bc[:, sl], pb[:])

            def gate_c(ib, y_row):
                if ib >= 4:
                    return
                sl = slice(ib * 512, (ib + 1) * 512)
                g_row = rtmp.tile([1, 512], FP32R, tag="g", name="g")
                nc.vector.tensor_scalar(g_row[:], y_row[:],
                                        0.5, 0.5, op0=ALU.mult, op1=ALU.add)
                pb2 = bps.tile([128, 512], FP32, tag="pb", name="pb")
                nc.tensor.matmul(pb2[:], ones_r[0:1, :], g_row[:],
                                 start=True, stop=True)
                nc.vector.tensor_tensor(dzp[:, sl], pb2[:], identrep[:],
                                        op=ALU.mult)

            # ---- projections (i-blocks 4..7 then 0..3, loaded in pairs) ----
            PROC = [4, 5, 6, 7, 0, 1, 2, 3]
            with (
                tc.tile_pool(name="wtiles", bufs=1) as wpool,
                tc.tile_pool(name="xslices", bufs=12) as xpool,
                tc.tile_pool(name="projps", bufs=6, space="PSUM") as ppool,
            ):
                # startup: packed weight DMAs + first x pair's h4-7 on the
                # gpsimd (SWDGE cast) queue, first x pair's h0-3 via sync
                # startup: small per-h weight DMAs interleaved with the
                # first block's x singles so the first Q group is fed at
                # the lowest latency; later blocks load x as 4 packed
                # [128, 2, 512] DMAs (host-repacked xq[128, 8, N]).
                lo0 = PROC[0] * 512
                wqr, wkr, xs0 = [], [], []
                for h in range(8):
                    wr = wpool.tile([128, A], FP32R, tag=f"wqr{h}", name=f"wqr{h}")
                    nc.gpsimd.dma_start(wr[:], wq[h * 128:(h + 1) * 128, :])
                    wqr.append(wr)
                    xt = xpool.tile([128, 2, 512], FP32R, tag="xr", name="xr")
                    nc.gpsimd.dma_start(xt[:, 0:1, :],
                                        xq[:, h:h + 1, lo0:lo0 + 512])
                    xs0.append(xt)

                # ---- constants (emitted after the startup DMAs so they
                # don't delay the first operand loads) ----
                io = cpool.tile([128, 128], mybir.dt.int32, tag="io", name="io")
                nc.gpsimd.iota(io[:], pattern=[[1, 128]], base=0,
                               channel_multiplier=-1)
                ident = cpool.tile([128, 128], FP32, tag="ident", name="ident")
                nc.vector.tensor_scalar(ident[:], io[:], 0, None, op0=ALU.is_equal)
                dneg = cpool.tile([128, 128], FP32, tag="dneg", name="dneg")
                nc.vector.tensor_scalar(dneg[:], ident[:], NEG, None, op0=ALU.mult)
                identrep = cpool.tile([128, 512], BF16, tag="idrep", name="idrep")
                for k in range(4):
                    nc.vector.tensor_copy(identrep[:, k * 128:(k + 1) * 128],
                                          ident[:])
                misc_r = cpool.tile([128, 18], FP32R, tag="miscr", name="miscr")
                nc.vector.tensor_copy(misc_r[:], misc_sb[:])
                ones_r = cpool.tile([1, 128], FP32R, tag="ones", name="ones")
                nc.vector.tensor_copy(ones_r[:], ones_f[:])

                prev = None
                y_prev = None
                for k, ib in enumerate(PROC):
                    sl = slice(ib * 512, (ib + 1) * 512)
                    if k == 0:
                        xs = [(xs0[h], 0) for h in range(8)]
                    else:
                        xs = []
                        for hp in range(4):
                            xt = xpool.tile([128, 2, 512], FP32R, tag="xr",
                                            name="xr")
                            nc.gpsimd.dma_start(
                                xt[:], xq[:, 2 * hp:2 * hp + 2, sl])
                            xs.append((xt, 0))
                            xs.append((xt, 1))
                    # deferred wk loads: not needed until the first K block
                    # (5th processed), so keep them off the gpsimd queue
                    # during the startup-critical x prefetch window
                    if k in (2, 3):
                        for h in range(4 * (k - 2), 4 * (k - 1)):
                            wr2 = wpool.tile([128, A], FP32R, tag=f"wkr{h}",
                                             name=f"wkr{h}")
                            nc.gpsimd.dma_start(wr2[:],
                                                wk[h * 128:(h + 1) * 128, :])
                            wkr.append(wr2)
                    for a in range(4):
                        pq = ppool.tile([128, 512], FP32, tag="ps", name="ps")
                        for h in range(8):
                            xt, m = xs[h]
                            nc.tensor.matmul(pq[:], wqr[h][:, a * 128:(a + 1) * 128],
                                             xt[:, m:m + 1, :],
                                             start=(h == 0), stop=(h == 7))
                        nc.scalar.activation(qwt[a][:, sl], pq[:],
                                             AF.Identity, bias=misc_sb[:, a:a + 1])
                        if prev is not None:
                            if a == 0:
                                y_prev = gate_a(prev)
                            elif a == 1:
                                gate_b(prev, y_prev)
                            elif a == 2:
                                gate_c(prev, y_prev)
                        if ib < 4:
                            pk = ppool.tile([128, 512], FP32, tag="ps", name="ps")
                            for h in range(8):
                                xt, m = xs[h]
                                nc.tensor.matmul(pk[:], wkr[h][:, a * 128:(a + 1) * 128],
                                                 xt[:, m:m + 1, :],
                                                 start=(h == 0), stop=(h == 7))
                            nc.scalar.activation(kwt[a][:, sl], pk[:],
                                                 AF.Identity, bias=misc_sb[:, 4 + a:5 + a])
                    prev = ib

            # ---- main loop over column tiles (output stays transposed).
            # The last pending gate (i-block 3) is emitted between t=0's
            # chunk groups for PE runway.
            with (
                tc.tile_pool(name="expp", bufs=4) as epool,
                tc.tile_pool(name="dsum", bufs=2) as dpool,
                tc.tile_pool(name="scoreps", bufs=3, space="PSUM") as sps,
            ):
                for t in range(16):
                    exp_t = epool.tile([128, N], BF16, tag="exp", name="exp")
                    dsum = dpool.tile([128, 5], FP32, tag="ds", name="ds")
                    dch = (t * 128) // 1024
                    for ch in range(4):
                        ps = sps.tile([128, 1024], FP32, tag="sc", name="sc")
                        for sub in range(2):
                            o = ch * 1024 + sub * 512
                            for a in range(4):
                                nc.tensor.matmul(ps[:, sub * 512:(sub + 1) * 512],
                                                 kwt[a][:, t * 128:(t + 1) * 128],
                                                 qwt[a][:, o:o + 512],
                                                 start=(a == 0), stop=(a == 3))
                        if ch == dch:
                            off = t * 128 - ch * 1024
                            nc.vector.tensor_add(ps[:, off:off + 128],
                                                 ps[:, off:off + 128], dneg[:])
                        if t == 15 and ch == 3:
                            # last chunk of the kernel: split the exp so the
                            # row sums (and thus the scale chain) start half
                            # a chunk earlier
                            nc.scalar.activation(exp_t[:, 3072:3584], ps[:, 0:512],
                                                 AF.Exp, accum_out=dsum[:, 3:4])
                            nc.scalar.activation(exp_t[:, 3584:4096], ps[:, 512:1024],
                                                 AF.Exp, accum_out=dsum[:, 4:5])
                        else:
                            nc.scalar.activation(exp_t[:, ch * 1024:(ch + 1) * 1024],
                                                 ps[:], AF.Exp,
                                                 accum_out=dsum[:, ch:ch + 1])
                        if t == 0 and prev is not None:
                            if ch == 0:
                                y_prev = gate_a(prev)
                            elif ch == 1:
                                gate_b(prev, y_prev)
                            elif ch == 2:
                                gate_c(prev, y_prev)
                                prev = None
                        # the x(1-g_i) factor is rcol-independent: apply it
                        # here, overlapped with the next chunk's matmuls, so
                        # only the fast x(1/denom) pass remains on the
                        # post-row-sum critical path
                        sl = slice(ch * 1024, (ch + 1) * 1024)
                        nc.vector.tensor_mul(exp_t[:, sl], exp_t[:, sl],
                                             g1m_bc[:, sl])
                    rsum = dpool.tile([128, 1], FP32, tag="r", name="r")
                    ncols = 5 if t == 15 else 4
                    nc.vector.tensor_reduce(rsum[:], dsum[:, 0:ncols],
                                            axis=mybir.AxisListType.X, op=ALU.add)
                    rcol = dpool.tile([128, 1], FP32, tag="r2", name="r2")
                    nc.vector.reciprocal_approx_fast(out=rcol[:], in_=rsum[:])
                    # last tile: diag chunk first so its extra dz add is off
                    # the final-DMA critical path
                    chorder = (dch, *(c for c in range(4) if c != dch)) if t == 15 \
                        else range(4)
                    if t < 15:
                        for ch in chorder:
                            sl = slice(ch * 1024, (ch + 1) * 1024)
                            nc.vector.tensor_scalar(exp_t[:, sl], exp_t[:, sl],
                                                    rcol[:], None, op0=ALU.mult)
                            if ch == dch:
                                nc.vector.tensor_add(exp_t[:, t * 128:(t + 1) * 128],
                                                     exp_t[:, t * 128:(t + 1) * 128],
                                                     dzp[:, t * 128:(t + 1) * 128])
                            nc.sync.dma_start(out[t * 128:(t + 1) * 128, sl],
                                              exp_t[:, sl])
                    else:
                        # final tile: 512-wide pieces + DMAs spread over both
                        # queues so the very last store leaves as early as
                        # possible (this chain is the kernel's end)
                        pieces = [2 * c + s for c in chorder for s in range(2)]
                        for n, pc in enumerate(pieces):
                            sl = slice(pc * 512, (pc + 1) * 512)
                            nc.vector.tensor_scalar(exp_t[:, sl], exp_t[:, sl],
                                                    rcol[:], None, op0=ALU.mult)
                            if pc == (t * 128) // 512:
                                nc.vector.tensor_add(
                                    exp_t[:, t * 128:(t + 1) * 128],
                                    exp_t[:, t * 128:(t + 1) * 128],
                                    dzp[:, t * 128:(t + 1) * 128])
                            eng = nc.sync if n % 2 == 0 else nc.gpsimd
                            eng.dma_start(out[t * 128:(t + 1) * 128, sl],
                                          exp_t[:, sl])
    nc.compile()
    return nc


def kernel(x, Wq, bq, Wk, bk, Wv, bv, _trace=False):
    x = np.asarray(x, dtype=np.float32)
    if "nc" not in _CACHE:
        _CACHE["nc"] = _build()
    nc = _CACHE["nc"]

    misc = np.zeros((128, 18), dtype=np.float32)
    misc[:, 0:4] = np.asarray(bq, np.float32).reshape(4, 128).T
    misc[:, 4:8] = np.asarray(bk, np.float32).reshape(4, 128).T
    wv_c = np.asarray(Wv, np.float32).reshape(4, 128).T
    misc[:, 8:16:2] = wv_c
    misc[:, 9:16:2] = wv_c
    # tanh path evaluates tanh(0.5*z + bv/2)
    misc[:, 16] = 0.5 * np.float32(np.asarray(bv).reshape(())[()])
    wq_np = np.ascontiguousarray(np.asarray(Wq, np.float32))
    wk_np = np.ascontiguousarray(np.asarray(Wk, np.float32))

    in_maps = []
    for c in range(8):
        b, h = c // 2, c % 2
        xT = x[b].T  # (H, N)
        if h != 0:
            xT = np.concatenate([xT[:, NSH:], xT[:, :NSH]], axis=1)
        # packed [128, 8, N]: xp[p, h, i] = xT[h*128+p, i]
        xqc = np.ascontiguousarray(xT.reshape(8, 128, N).transpose(1, 0, 2))
        in_maps.append({"xq": xqc, "wq": wq_np, "wk": wk_np, "misc": misc})

    res = run_bass_kernel_spmd(nc, in_maps, list(range(8)), trace=_trace)

    outp = np.empty((B, N, N), dtype=np.float32)
    for c in range(8):
        b, h = c // 2, c % 2
        O = np.asarray(res.results[c]["out"], dtype=np.float32).T  # (N i_perm, NSH j)
        js = slice(h * NSH, (h + 1) * NSH)
        outp[b, h * NSH:(h + 1) * NSH, js] = O[:NSH]
        outp[b, (1 - h) * NSH:(2 - h) * NSH, js] = O[NSH:]
    if _trace:
        return outp, res
    return outp


# revision 39
# speedup vs baseline: 1.0106x; 1.0106x over previous
"""GatedAttention Trainium2 kernel.

Math (per batch b):
  Qw = x @ Wq + bq            (N, A)
  Kw = x @ Wk + bk            (N, A)
  g  = sigmoid(Qw @ Wv + bv)  (N,)
  S  = Qw @ Kw^T, diag -> -inf
  P  = softmax(S, axis=0)     (column softmax)
  out = (1-g)[:,None] * P + g[:,None] * I

Sharding: 8 cores = 4 batches x 2 column-halves of the score matrix.
Column softmax is independent per column, so no cross-core reduction.

Device layout: scores computed transposed, sT[j, i] tiles (j on partitions)
so the softmax reduction over i is a free-axis reduction. The i axis is
host-permuted so each core's diagonal block sits at i in [0, 2048) —
this keeps the program identical across cores (pure SPMD).

Pipeline per core:
  Projections (per 512-wide i-block, processed 4..7 then 0..3 in pairs):
  x arrives as [128, 1024] pair-tiles via SWDGE cast-DMA straight into
  fp32r (rounds in flight); weights are host-packed [128, 8, A] and land
  in two packed cast-DMAs each. The first block's Q accumulation is
  split into h0-3/h4-7 brackets fed by the sync and gpsimd DMA queues in
  parallel so the PE starts ~5us earlier and HAM never re-throttles.
  The gate (z = Qw @ Wv, g = 0.5+0.5*tanh(z/2); tanh shares the exp ACT
  table set) is software-pipelined one block behind, its PE ops
  interleaved between the next block's Q-groups so the ACT/DVE chain
  never stalls the strict-FIFO PE queue; the final gate lands inside
  the main loop's first tile. Diagonal planes dzp = ident * g are
  precomputed.
  Main loop per 128-column tile t: score matmuls -> PSUM, diag -1e30,
  exp (bf16 out, fp32 row-sum accum), 1/sum via fast reciprocal, bf16
  tensor_scalar (x 1/denom) + tensor_tensor (x (1-g_i)) passes, diag
  adds dzp, output streams out per 1024-wide chunk (bf16, host casts
  back to fp32).
"""
import numpy as np

import concourse.bacc as bacc
import concourse.mybir as mybir
import concourse.tile as tile
from concourse.bass_utils import run_bass_kernel_spmd

FP32 = mybir.dt.float32
FP32R = mybir.dt.float32r
BF16 = mybir.dt.bfloat16
AF = mybir.ActivationFunctionType
ALU = mybir.AluOpType

B, N, H, A = 4, 4096, 1024, 512
NSH = N // 2          # per-core column shard
NEG = -1.0e30

_CACHE = {}


def _build():
    nc = bacc.Bacc("TRN2", target_bir_lowering=False, debug=False, num_devices=8)
    xq = nc.dram_tensor("xq", [128, 8, N], FP32, kind="ExternalInput").ap()
    wq = nc.dram_tensor("wq", [H, A], FP32, kind="ExternalInput").ap()
    wk = nc.dram_tensor("wk", [H, A], FP32, kind="ExternalInput").ap()
    misc = nc.dram_tensor("misc", [128, 18], FP32, kind="ExternalInput").ap()
    out = nc.dram_tensor("out", [NSH, N], BF16, kind="ExternalOutput").ap()

    with tile.TileContext(nc) as tc:
        with (
            tc.tile_pool(name="const", bufs=1) as cpool,
            tc.tile_pool(name="proj_out", bufs=1) as qkpool,
            tc.tile_pool(name="gate", bufs=1) as gpool,
            tc.tile_pool(name="zrowps", bufs=1, space="PSUM") as zpool,
            tc.tile_pool(name="bcps", bufs=1, space="PSUM") as bps,
            tc.tile_pool(name="rowtmp", bufs=1) as rtmp,
        ):
            misc_sb = cpool.tile([128, 18], FP32, tag="misc", name="misc")
            nc.sync.dma_start(misc_sb[:], misc)

            # ---- PE warmup: dependency-free dummy matmuls issued while the
            # first operand DMAs are in flight, so the HAM activity window
            # opens before real work arrives (otherwise the first ~60 real
            # matmuls run at the cold 1.2 GHz rate). fp32 x fp32 on a
            # memset row — no fp32r rounding constraints involved.
            ones_f = cpool.tile([1, 512], FP32, tag="onesf", name="onesf")
            nc.vector.memset(ones_f[:], 1.0)
            warm = bps.tile([128, 512], FP32, tag="pb", name="warm")
            for _ in range(4):
                nc.tensor.matmul(warm[:], ones_f[0:1, 0:128], ones_f[0:1, :],
                                 start=True, stop=True)

            # ---- persistent projection outputs (fp32r) + gate planes ----
            qwt = [qkpool.tile([128, N], FP32R, tag=f"qwt{a}", name=f"qwt{a}") for a in range(4)]
            kwt = [qkpool.tile([128, NSH], FP32R, tag=f"kwt{a}", name=f"kwt{a}") for a in range(4)]
            g1m_bc = gpool.tile([128, N], BF16, tag="g1mbc", name="g1mbc")
            dzp = gpool.tile([128, NSH], BF16, tag="dzp", name="dzp")

            # gate stages, software-pipelined one i-block behind the
            # projections (stage A: z matmuls + tanh; B: (1-g) row +
            # broadcast; C: g row + diag plane)
            def gate_a(ib):
                sl = slice(ib * 512, (ib + 1) * 512)
                pzc = zpool.tile([2, 512], FP32, tag="zr", name="zr")
                for a in range(4):
                    nc.tensor.matmul(pzc[:], misc_r[:, 8 + 2 * a:10 + 2 * a],
                                     qwt[a][:, sl], start=(a == 0), stop=(a == 3))
                y_row = rtmp.tile([1, 512], FP32, tag="y", name="y")
                nc.scalar.activation(y_row[:], pzc[0:1, :], AF.Tanh,
                                     bias=misc_sb[0:1, 16:17], scale=0.5)
                return y_row

            def gate_b(ib, y_row):
                sl = slice(ib * 512, (ib + 1) * 512)
                g1m_row = rtmp.tile([1, 512], FP32R, tag="g1m", name="g1m")
                nc.vector.tensor_scalar(g1m_row[:], y_row[:],
                                        -0.5, 0.5, op0=ALU.mult, op1=ALU.add)
                pb = bps.tile([128, 512], FP32, tag="pb", name="pb")
                nc.tensor.matmul(pb[:], ones_r[0:1, :], g1m_row[:],
                                 start=True, stop=True)
                nc.vector.tensor_copy(g1m_bc[:, sl], pb[:])

            def gate_c(ib, y_row):
                if ib >= 4:
                    return
                sl = slice(ib * 512, (ib + 1) * 512)
                g_row = rtmp.tile([1, 512], FP32R, tag="g", name="g")
                nc.vector.tensor_scalar(g_row[:], y_row[:],
                                        0.5, 0.5, op0=ALU.mult, op1=ALU.add)
                pb2 = bps.tile([128, 512], FP32, tag="pb", name="pb")
                nc.tensor.matmul(pb2[:], ones_r[0:1, :], g_row[:],
                                 start=True, stop=True)
                nc.vector.tensor_tensor(dzp[:, sl], pb2[:], identrep[:],
                                        op=ALU.mult)

            # ---- projections (i-blocks 4..7 then 0..3, loaded in pairs) ----
            PROC = [4, 5, 6, 7, 0, 1, 2, 3]
            with (
                tc.tile_pool(name="wtiles", bufs=1) as wpool,
                tc.tile_pool(name="xslices", bufs=12) as xpool,
                tc.tile_pool(name="projps", bufs=6, space="PSUM") as ppool,
            ):
                # startup: packed weight DMAs + first x pair's h4-7 on the
                # gpsimd (SWDGE cast) queue, first x pair's h0-3 via sync
                # startup: small per-h weight DMAs interleaved with the
                # first block's x singles so the first Q group is fed at
                # the lowest latency; later blocks load x as 4 packed
                # [128, 2, 512] DMAs (host-repacked xq[128, 8, N]).
                lo0 = PROC[0] * 512
                wqr, wkr, xs0 = [], [], []
                for h in range(8):
                    wr = wpool.tile([128, A], FP32R, tag=f"wqr{h}", name=f"wqr{h}")
                    nc.gpsimd.dma_start(wr[:], wq[h * 128:(h + 1) * 128, :])
                    wqr.append(wr)
                    xt = xpool.tile([128, 2, 512], FP32R, tag="xr", name="xr")
                    nc.gpsimd.dma_start(xt[:, 0:1, :],
                                        xq[:, h:h + 1, lo0:lo0 + 512])
                    xs0.append(xt)

                # ---- constants (emitted after the startup DMAs so they
                # don't delay the first operand loads) ----
                io = cpool.tile([128, 128], mybir.dt.int32, tag="io", name="io")
                nc.gpsimd.iota(io[:], pattern=[[1, 128]], base=0,
                               channel_multiplier=-1)
                ident = cpool.tile([128, 128], FP32, tag="ident", name="ident")
                nc.vector.tensor_scalar(ident[:], io[:], 0, None, op0=ALU.is_equal)
                dneg = cpool.tile([128, 128], FP32, tag="dneg", name="dneg")
                nc.vector.tensor_scalar(dneg[:], ident[:], NEG, None, op0=ALU.mult)
                identrep = cpool.tile([128, 512], BF16, tag="idrep", name="idrep")
                for k in range(4):
                    nc.vector.tensor_copy(identrep[:, k * 128:(k + 1) * 128],
                                          ident[:])
                misc_r = cpool.tile([128, 18], FP32R, tag="miscr", name="miscr")
                nc.vector.tensor_copy(misc_r[:], misc_sb[:])
                ones_r = cpool.tile([1, 128], FP32R, tag="ones", name="ones")
                nc.vector.tensor_copy(ones_r[:], ones_f[0:1, 0:128])

                prev = None
                y_prev = None
                for k, ib in enumerate(PROC):
                    sl = slice(ib * 512, (ib + 1) * 512)
                    if k == 0:
                        xs = [(xs0[h], 0) for h in range(8)]
                    else:
                        xs = []
                        for hp in range(4):
                            xt = xpool.tile([128, 2, 512], FP32R, tag="xr",
                                            name="xr")
                            nc.gpsimd.dma_start(
                                xt[:], xq[:, 2 * hp:2 * hp + 2, sl])
                            xs.append((xt, 0))
                            xs.append((xt, 1))
                    # deferred wk loads: not needed until the first K block
                    # (5th processed), so keep them off the gpsimd queue
                    # during the startup-critical x prefetch window
                    if k in (2, 3):
                        for h in range(4 * (k - 2), 4 * (k - 1)):
                            wr2 = wpool.tile([128, A], FP32R, tag=f"wkr{h}",
                                             name=f"wkr{h}")
                            nc.gpsimd.dma_start(wr2[:],
                                                wk[h * 128:(h + 1) * 128, :])
                            wkr.append(wr2)
                    for a in range(4):
                        pq = ppool.tile([128, 512], FP32, tag="ps", name="ps")
                        for h in range(8):
                            xt, m = xs[h]
                            nc.tensor.matmul(pq[:], wqr[h][:, a * 128:(a + 1) * 128],
                                             xt[:, m:m + 1, :],
                                             start=(h == 0), stop=(h == 7))
                        nc.scalar.activation(qwt[a][:, sl], pq[:],
                                             AF.Identity, bias=misc_sb[:, a:a + 1])
                        if prev is not None:
                            if a == 0:
                                y_prev = gate_a(prev)
                            elif a == 2:
                                gate_b(prev, y_prev)
                            elif a == 3:
                                gate_c(prev, y_prev)
                        if ib < 4:
                            pk = ppool.tile([128, 512], FP32, tag="ps", name="ps")
                            for h in range(8):
                                xt, m = xs[h]
                                nc.tensor.matmul(pk[:], wkr[h][:, a * 128:(a + 1) * 128],
                                                 xt[:, m:m + 1, :],
                                                 start=(h == 0), stop=(h == 7))
                            nc.scalar.activation(kwt[a][:, sl], pk[:],
                                                 AF.Identity, bias=misc_sb[:, 4 + a:5 + a])
                    prev = ib

            # ---- main loop over column tiles (output stays transposed).
            # The last pending gate (i-block 3) is emitted between t=0's
            # chunk groups for PE runway.
            with (
                tc.tile_pool(name="expp", bufs=4) as epool,
                tc.tile_pool(name="dsum", bufs=2) as dpool,
                tc.tile_pool(name="scoreps", bufs=3, space="PSUM") as sps,
            ):
                for t in range(16):
                    exp_t = epool.tile([128, N], BF16, tag="exp", name="exp")
                    dsum = dpool.tile([128, 5], FP32, tag="ds", name="ds")
                    dch = (t * 128) // 1024
                    for ch in range(4):
                        ps = sps.tile([128, 1024], FP32, tag="sc", name="sc")
                        # a-outer: each kwt stationary tile feeds two
                        # consecutive matmuls (the two 512-wide sub-banks)
                        for a in range(4):
                            for sub in range(2):
                                o = ch * 1024 + sub * 512
                                nc.tensor.matmul(ps[:, sub * 512:(sub + 1) * 512],
                                                 kwt[a][:, t * 128:(t + 1) * 128],
                                                 qwt[a][:, o:o + 512],
                                                 start=(a == 0), stop=(a == 3))
                        if ch == dch:
                            off = t * 128 - ch * 1024
                            nc.vector.tensor_add(ps[:, off:off + 128],
                                                 ps[:, off:off + 128], dneg[:])
                        if t == 15 and ch == 3:
                            # last chunk of the kernel: split the exp so the
                            # row sums (and thus the scale chain) start half
                            # a chunk earlier
                            nc.scalar.activation(exp_t[:, 3072:3584], ps[:, 0:512],
                                                 AF.Exp, accum_out=dsum[:, 3:4])
                            nc.scalar.activation(exp_t[:, 3584:4096], ps[:, 512:1024],
                                                 AF.Exp, accum_out=dsum[:, 4:5])
                        else:
                            nc.scalar.activation(exp_t[:, ch * 1024:(ch + 1) * 1024],
                                                 ps[:], AF.Exp,
                                                 accum_out=dsum[:, ch:ch + 1])
                        if t == 0 and prev is not None:
                            if ch == 0:
                                y_prev = gate_a(prev)
                            elif ch == 1:
                                gate_b(prev, y_prev)
                            elif ch == 2:
                                gate_c(prev, y_prev)
                                prev = None
                        # the x(1-g_i) factor is rcol-independent: apply it
                        # here, overlapped with the next chunk's matmuls, so
                        # only the fast x(1/denom) pass remains on the
                        # post-row-sum critical path
                        sl = slice(ch * 1024, (ch + 1) * 1024)
                        nc.vector.tensor_mul(exp_t[:, sl], exp_t[:, sl],
                                             g1m_bc[:, sl])
                    rsum = dpool.tile([128, 1], FP32, tag="r", name="r")
                    ncols = 5 if t == 15 else 4
                    nc.vector.tensor_reduce(rsum[:], dsum[:, 0:ncols],
                                            axis=mybir.AxisListType.X, op=ALU.add)
                    rcol = dpool.tile([128, 1], FP32, tag="r2", name="r2")
                    nc.vector.reciprocal_approx_fast(out=rcol[:], in_=rsum[:])
                    # last tile: diag chunk first so its extra dz add is off
                    # the final-DMA critical path
                    chorder = (dch, *(c for c in range(4) if c != dch)) if t == 15 \
                        else range(4)
                    for ch in chorder:
                        sl = slice(ch * 1024, (ch + 1) * 1024)
                        nc.vector.tensor_scalar(exp_t[:, sl], exp_t[:, sl],
                                                rcol[:], None, op0=ALU.mult)
                        if ch == dch:
                            nc.vector.tensor_add(exp_t[:, t * 128:(t + 1) * 128],
                                                 exp_t[:, t * 128:(t + 1) * 128],
                                                 dzp[:, t * 128:(t + 1) * 128])
                        nc.sync.dma_start(out[t * 128:(t + 1) * 128, sl],
                                          exp_t[:, sl])
    nc.compile()
    return nc


def kernel(x, Wq, bq, Wk, bk, Wv, bv, _trace=False):
    x = np.asarray(x, dtype=np.float32)
    if "nc" not in _CACHE:
        _CACHE["nc"] = _build()
    nc = _CACHE["nc"]

    misc = np.zeros((128, 18), dtype=np.float32)
    misc[:, 0:4] = np.asarray(bq, np.float32).reshape(4, 128).T
    misc[:, 4:8] = np.asarray(bk, np.float32).reshape(4, 128).T
    wv_c = np.asarray(Wv, np.float32).reshape(4, 128).T
    misc[:, 8:16:2] = wv_c
    misc[:, 9:16:2] = wv_c
    # tanh path evaluates tanh(0.5*z + bv/2)
    misc[:, 16] = 0.5 * np.float32(np.asarray(bv).reshape(())[()])
    wq_np = np.ascontiguousarray(np.asarray(Wq, np.float32))
    wk_np = np.ascontiguousarray(np.asarray(Wk, np.float32))

    in_maps = []
    for c in range(8):
        b, h = c // 2, c % 2
        xT = x[b].T  # (H, N)
        if h != 0:
            xT = np.concatenate([xT[:, NSH:], xT[:, :NSH]], axis=1)
        # packed [128, 8, N]: xp[p, h, i] = xT[h*128+p, i]
        xqc = np.ascontiguousarray(xT.reshape(8, 128, N).transpose(1, 0, 2))
        in_maps.append({"xq": xqc, "wq": wq_np, "wk": wk_np, "misc": misc})

    res = run_bass_kernel_spmd(nc, in_maps, list(range(8)), trace=_trace)

    outp = np.empty((B, N, N), dtype=np.float32)
    for c in range(8):
        b, h = c // 2, c % 2
        O = np.asarray(res.results[c]["out"], dtype=np.float32).T  # (N i_perm, NSH j)
        js = slice(h * NSH, (h + 1) * NSH)
        outp[b, h * NSH:(h + 1) * NSH, js] = O[:NSH]
        outp[b, (1 - h) * NSH:(2 - h) * NSH, js] = O[NSH:]
    if _trace:
        return outp, res
    return outp


# revision 43
# speedup vs baseline: 1.0110x; 1.0004x over previous
"""GatedAttention Trainium2 kernel.

Math (per batch b):
  Qw = x @ Wq + bq            (N, A)
  Kw = x @ Wk + bk            (N, A)
  g  = sigmoid(Qw @ Wv + bv)  (N,)
  S  = Qw @ Kw^T, diag -> -inf
  P  = softmax(S, axis=0)     (column softmax)
  out = (1-g)[:,None] * P + g[:,None] * I

Sharding: 8 cores = 4 batches x 2 column-halves of the score matrix.
Column softmax is independent per column, so no cross-core reduction.

Device layout: scores computed transposed, sT[j, i] tiles (j on partitions)
so the softmax reduction over i is a free-axis reduction. The i axis is
host-permuted so each core's diagonal block sits at i in [0, 2048) —
this keeps the program identical across cores (pure SPMD).

Pipeline per core:
  Projections (per 512-wide i-block, processed 4..7 then 0..3 in pairs):
  x arrives as [128, 1024] pair-tiles via SWDGE cast-DMA straight into
  fp32r (rounds in flight); weights are host-packed [128, 8, A] and land
  in two packed cast-DMAs each. The first block's Q accumulation is
  split into h0-3/h4-7 brackets fed by the sync and gpsimd DMA queues in
  parallel so the PE starts ~5us earlier and HAM never re-throttles.
  The gate (z = Qw @ Wv, g = 0.5+0.5*tanh(z/2); tanh shares the exp ACT
  table set) is software-pipelined one block behind, its PE ops
  interleaved between the next block's Q-groups so the ACT/DVE chain
  never stalls the strict-FIFO PE queue; the final gate lands inside
  the main loop's first tile. Diagonal planes dzp = ident * g are
  precomputed.
  Main loop per 128-column tile t: score matmuls -> PSUM, diag -1e30,
  exp (bf16 out, fp32 row-sum accum), 1/sum via fast reciprocal, bf16
  tensor_scalar (x 1/denom) + tensor_tensor (x (1-g_i)) passes, diag
  adds dzp, output streams out per 1024-wide chunk (bf16, host casts
  back to fp32).
"""
import numpy as np

import concourse.bacc as bacc
import concourse.mybir as mybir
import concourse.tile as tile
from concourse.bass_utils import run_bass_kernel_spmd

FP32 = mybir.dt.float32
FP32R = mybir.dt.float32r
BF16 = mybir.dt.bfloat16
AF = mybir.ActivationFunctionType
ALU = mybir.AluOpType

B, N, H, A = 4, 4096, 1024, 512
NSH = N // 2          # per-core column shard
NEG = -1.0e30

_CACHE = {}


def _build():
    nc = bacc.Bacc("TRN2", target_bir_lowering=False, debug=False, num_devices=8)
    xq = nc.dram_tensor("xq", [128, 8, N], FP32, kind="ExternalInput").ap()
    wq = nc.dram_tensor("wq", [H, A], FP32, kind="ExternalInput").ap()
    wk = nc.dram_tensor("wk", [H, A], FP32, kind="ExternalInput").ap()
    misc = nc.dram_tensor("misc", [128, 18], FP32, kind="ExternalInput").ap()
    out = nc.dram_tensor("out", [NSH, N], BF16, kind="ExternalOutput").ap()

    with tile.TileContext(nc) as tc:
        with (
            tc.tile_pool(name="const", bufs=1) as cpool,
            tc.tile_pool(name="proj_out", bufs=1) as qkpool,
            tc.tile_pool(name="gate", bufs=1) as gpool,
            tc.tile_pool(name="zrowps", bufs=1, space="PSUM") as zpool,
            tc.tile_pool(name="bcps", bufs=1, space="PSUM") as bps,
            tc.tile_pool(name="rowtmp", bufs=1) as rtmp,
        ):
            misc_sb = cpool.tile([128, 18], FP32, tag="misc", name="misc")
            nc.sync.dma_start(misc_sb[:], misc)

            # ---- PE warmup: dependency-free dummy matmuls issued while the
            # first operand DMAs are in flight, so the HAM activity window
            # opens before real work arrives (otherwise the first ~60 real
            # matmuls run at the cold 1.2 GHz rate). fp32 x fp32 on a
            # memset row — no fp32r rounding constraints involved.
            ones_f = cpool.tile([1, 128], FP32, tag="onesf", name="onesf")
            nc.vector.memset(ones_f[:], 1.0)
            warm = bps.tile([128, 512], FP32, tag="pb", name="warm")
            for _ in range(12):
                nc.tensor.matmul(warm[:, 0:128], ones_f[0:1, :], ones_f[0:1, :],
                                 start=True, stop=True)

            # ---- persistent projection outputs (fp32r) + gate planes ----
            qwt = [qkpool.tile([128, N], FP32R, tag=f"qwt{a}", name=f"qwt{a}") for a in range(4)]
            kwt = [qkpool.tile([128, NSH], FP32R, tag=f"kwt{a}", name=f"kwt{a}") for a in range(4)]
            g1m_bc = gpool.tile([128, N], BF16, tag="g1mbc", name="g1mbc")
            dzp = gpool.tile([128, NSH], BF16, tag="dzp", name="dzp")

            # gate stages, software-pipelined one i-block behind the
            # projections (stage A: z matmuls + tanh; B: (1-g) row +
            # broadcast; C: g row + diag plane)
            def gate_a(ib):
                sl = slice(ib * 512, (ib + 1) * 512)
                pzc = zpool.tile([2, 512], FP32, tag="zr", name="zr")
                for a in range(4):
                    nc.tensor.matmul(pzc[:], misc_r[:, 8 + 2 * a:10 + 2 * a],
                                     qwt[a][:, sl], start=(a == 0), stop=(a == 3))
                y_row = rtmp.tile([1, 512], FP32, tag="y", name="y")
                nc.scalar.activation(y_row[:], pzc[0:1, :], AF.Tanh,
                                     bias=misc_sb[0:1, 16:17], scale=0.5)
                return y_row

            def gate_b(ib, y_row):
                sl = slice(ib * 512, (ib + 1) * 512)
                g1m_row = rtmp.tile([1, 512], FP32R, tag="g1m", name="g1m")
                nc.vector.tensor_scalar(g1m_row[:], y_row[:],
                                        -0.5, 0.5, op0=ALU.mult, op1=ALU.add)
                pb = bps.tile([128, 512], FP32, tag="pb", name="pb")
                nc.tensor.matmul(pb[:], ones_r[0:1, :], g1m_row[:],
                                 start=True, stop=True)
                nc.vector.tensor_copy(g1m_bc[:, sl], pb[:])

            def gate_c(ib, y_row):
                if ib >= 4:
                    return
                sl = slice(ib * 512, (ib + 1) * 512)
                g_row = rtmp.tile([1, 512], FP32R, tag="g", name="g")
                nc.vector.tensor_scalar(g_row[:], y_row[:],
                                        0.5, 0.5, op0=ALU.mult, op1=ALU.add)
                pb2 = bps.tile([128, 512], FP32, tag="pb", name="pb")
                nc.tensor.matmul(pb2[:], ones_r[0:1, :], g_row[:],
                                 start=True, stop=True)
                nc.vector.tensor_tensor(dzp[:, sl], pb2[:], identrep[:],
                                        op=ALU.mult)

            # ---- projections (i-blocks 4..7 then 0..3, loaded in pairs) ----
            PROC = [4, 5, 6, 7, 0, 1, 2, 3]
            with (
                tc.tile_pool(name="wtiles", bufs=1) as wpool,
                tc.tile_pool(name="xslices", bufs=12) as xpool,
                tc.tile_pool(name="projps", bufs=6, space="PSUM") as ppool,
            ):
                # startup: packed weight DMAs + first x pair's h4-7 on the
                # gpsimd (SWDGE cast) queue, first x pair's h0-3 via sync
                # startup: small per-h weight DMAs interleaved with the
                # first block's x singles so the first Q group is fed at
                # the lowest latency; later blocks load x as 4 packed
                # [128, 2, 512] DMAs (host-repacked xq[128, 8, N]).
                lo0 = PROC[0] * 512
                wqr, wkr, xs0 = [], [], []
                for h in range(8):
                    wr = wpool.tile([128, A], FP32R, tag=f"wqr{h}", name=f"wqr{h}")
                    nc.gpsimd.dma_start(wr[:], wq[h * 128:(h + 1) * 128, :])
                    wqr.append(wr)
                    xt = xpool.tile([128, 2, 512], FP32R, tag="xr", name="xr")
                    nc.gpsimd.dma_start(xt[:, 0:1, :],
                                        xq[:, h:h + 1, lo0:lo0 + 512])
                    xs0.append(xt)

                # ---- constants (emitted after the startup DMAs so they
                # don't delay the first operand loads) ----
                io = cpool.tile([128, 128], mybir.dt.int32, tag="io", name="io")
                nc.gpsimd.iota(io[:], pattern=[[1, 128]], base=0,
                               channel_multiplier=-1)
                ident = cpool.tile([128, 128], FP32, tag="ident", name="ident")
                nc.vector.tensor_scalar(ident[:], io[:], 0, None, op0=ALU.is_equal)
                dneg = cpool.tile([128, 128], FP32, tag="dneg", name="dneg")
                nc.vector.tensor_scalar(dneg[:], ident[:], NEG, None, op0=ALU.mult)
                identrep = cpool.tile([128, 512], BF16, tag="idrep", name="idrep")
                for k in range(4):
                    nc.vector.tensor_copy(identrep[:, k * 128:(k + 1) * 128],
                                          ident[:])
                misc_r = cpool.tile([128, 18], FP32R, tag="miscr", name="miscr")
                nc.vector.tensor_copy(misc_r[:], misc_sb[:])
                ones_r = cpool.tile([1, 128], FP32R, tag="ones", name="ones")
                nc.vector.tensor_copy(ones_r[:], ones_f[:])

                prev = None
                y_prev = None
                for k, ib in enumerate(PROC):
                    sl = slice(ib * 512, (ib + 1) * 512)
                    if k == 0:
                        xs = [(xs0[h], 0) for h in range(8)]
                    else:
                        xs = []
                        for hp in range(4):
                            xt = xpool.tile([128, 2, 512], FP32R, tag="xr",
                                            name="xr")
                            nc.gpsimd.dma_start(
                                xt[:], xq[:, 2 * hp:2 * hp + 2, sl])
                            xs.append((xt, 0))
                            xs.append((xt, 1))
                    # deferred wk loads: not needed until the first K block
                    # (5th processed), so keep them off the gpsimd queue
                    # during the startup-critical x prefetch window
                    if k in (2, 3):
                        for h in range(4 * (k - 2), 4 * (k - 1)):
                            wr2 = wpool.tile([128, A], FP32R, tag=f"wkr{h}",
                                             name=f"wkr{h}")
                            nc.gpsimd.dma_start(wr2[:],
                                                wk[h * 128:(h + 1) * 128, :])
                            wkr.append(wr2)
                    for a in range(4):
                        pq = ppool.tile([128, 512], FP32, tag="ps", name="ps")
                        for h in range(8):
                            xt, m = xs[h]
                            nc.tensor.matmul(pq[:], wqr[h][:, a * 128:(a + 1) * 128],
                                             xt[:, m:m + 1, :],
                                             start=(h == 0), stop=(h == 7))
                        nc.scalar.activation(qwt[a][:, sl], pq[:],
                                             AF.Identity, bias=misc_sb[:, a:a + 1])
                        if prev is not None:
                            if a == 0:
                                y_prev = gate_a(prev)
                            elif a == 1:
                                gate_b(prev, y_prev)
                            elif a == 2:
                                gate_c(prev, y_prev)
                        if ib < 4:
                            pk = ppool.tile([128, 512], FP32, tag="ps", name="ps")
                            for h in range(8):
                                xt, m = xs[h]
                                nc.tensor.matmul(pk[:], wkr[h][:, a * 128:(a + 1) * 128],
                                                 xt[:, m:m + 1, :],
                                                 start=(h == 0), stop=(h == 7))
                            nc.scalar.activation(kwt[a][:, sl], pk[:],
                                                 AF.Identity, bias=misc_sb[:, 4 + a:5 + a])
                    prev = ib

            # ---- main loop over column tiles (output stays transposed).
            # The last pending gate (i-block 3) is emitted between t=0's
            # chunk groups for PE runway.
            with (
                tc.tile_pool(name="expp", bufs=4) as epool,
                tc.tile_pool(name="dsum", bufs=2) as dpool,
                tc.tile_pool(name="scoreps", bufs=3, space="PSUM") as sps,
            ):
                for t in range(16):
                    exp_t = epool.tile([128, N], BF16, tag="exp", name="exp")
                    dsum = dpool.tile([128, 5], FP32, tag="ds", name="ds")
                    dch = (t * 128) // 1024
                    for ch in range(4):
                        ps = sps.tile([128, 1024], FP32, tag="sc", name="sc")
                        for sub in range(2):
                            o = ch * 1024 + sub * 512
                            for a in range(4):
                                nc.tensor.matmul(ps[:, sub * 512:(sub + 1) * 512],
                                                 kwt[a][:, t * 128:(t + 1) * 128],
                                                 qwt[a][:, o:o + 512],
                                                 start=(a == 0), stop=(a == 3))
                        if ch == dch:
                            off = t * 128 - ch * 1024
                            nc.vector.tensor_add(ps[:, off:off + 128],
                                                 ps[:, off:off + 128], dneg[:])
                        if t == 15 and ch == 3:
                            # last chunk of the kernel: split the exp so the
                            # row sums (and thus the scale chain) start half
                            # a chunk earlier
                            nc.scalar.activation(exp_t[:, 3072:3584], ps[:, 0:512],
                                                 AF.Exp, accum_out=dsum[:, 3:4])
                            nc.scalar.activation(exp_t[:, 3584:4096], ps[:, 512:1024],
                                                 AF.Exp, accum_out=dsum[:, 4:5])
                        else:
                            nc.scalar.activation(exp_t[:, ch * 1024:(ch + 1) * 1024],
                                                 ps[:], AF.Exp,
                                                 accum_out=dsum[:, ch:ch + 1])
                        if t == 0 and prev is not None:
                            if ch == 0:
                                y_prev = gate_a(prev)
                            elif ch == 1:
                                gate_b(prev, y_prev)
                            elif ch == 2:
                                gate_c(prev, y_prev)
                                prev = None
                        # the x(1-g_i) factor is rcol-independent: apply it
                        # here, overlapped with the next chunk's matmuls, so
                        # only the fast x(1/denom) pass remains on the
                        # post-row-sum critical path
                        sl = slice(ch * 1024, (ch + 1) * 1024)
                        nc.vector.tensor_mul(exp_t[:, sl], exp_t[:, sl],
                                             g1m_bc[:, sl])
                    rsum = dpool.tile([128, 1], FP32, tag="r", name="r")
                    ncols = 5 if t == 15 else 4
                    nc.vector.tensor_reduce(rsum[:], dsum[:, 0:ncols],
                                            axis=mybir.AxisListType.X, op=ALU.add)
                    rcol = dpool.tile([128, 1], FP32, tag="r2", name="r2")
                    nc.vector.reciprocal_approx_fast(out=rcol[:], in_=rsum[:])
                    # last tile: diag chunk first so its extra dz add is off
                    # the final-DMA critical path
                    chorder = (dch, *(c for c in range(4) if c != dch)) if t == 15 \
                        else range(4)
                    for ch in chorder:
                        sl = slice(ch * 1024, (ch + 1) * 1024)
                        nc.vector.tensor_scalar(exp_t[:, sl], exp_t[:, sl],
                                                rcol[:], None, op0=ALU.mult)
                        if ch == dch:
                            nc.vector.tensor_add(exp_t[:, t * 128:(t + 1) * 128],
                                                 exp_t[:, t * 128:(t + 1) * 128],
                                                 dzp[:, t * 128:(t + 1) * 128])
                        nc.sync.dma_start(out[t * 128:(t + 1) * 128, sl],
                                          exp_t[:, sl])
    nc.compile()
    return nc


def kernel(x, Wq, bq, Wk, bk, Wv, bv, _trace=False):
    x = np.asarray(x, dtype=np.float32)
    if "nc" not in _CACHE:
        _CACHE["nc"] = _build()
    nc = _CACHE["nc"]

    misc = np.zeros((128, 18), dtype=np.float32)
    misc[:, 0:4] = np.asarray(bq, np.float32).reshape(4, 128).T
    misc[:, 4:8] = np.asarray(bk, np.float32).reshape(4, 128).T
    wv_c = np.asarray(Wv, np.float32).reshape(4, 128).T
    misc[:, 8:16:2] = wv_c
    misc[:, 9:16:2] = wv_c
    # tanh path evaluates tanh(0.5*z + bv/2)
    misc[:, 16] = 0.5 * np.float32(np.asarray(bv).reshape(())[()])
    wq_np = np.ascontiguousarray(np.asarray(Wq, np.float32))
    wk_np = np.ascontiguousarray(np.asarray(Wk, np.float32))

    in_maps = []
    for c in range(8):
        b, h = c // 2, c % 2
        xT = x[b].T  # (H, N)
        if h != 0:
            xT = np.concatenate([xT[:, NSH:], xT[:, :NSH]], axis=1)
        # packed [128, 8, N]: xp[p, h, i] = xT[h*128+p, i]
        xqc = np.ascontiguousarray(xT.reshape(8, 128, N).transpose(1, 0, 2))
        in_maps.append({"xq": xqc, "wq": wq_np, "wk": wk_np, "misc": misc})

    res = run_bass_kernel_spmd(nc, in_maps, list(range(8)), trace=_trace)

    outp = np.empty((B, N, N), dtype=np.float32)
    for c in range(8):
        b, h = c // 2, c % 2
        O = np.asarray(res.results[c]["out"], dtype=np.float32).T  # (N i_perm, NSH j)
        js = slice(h * NSH, (h + 1) * NSH)
        outp[b, h * NSH:(h + 1) * NSH, js] = O[:NSH]
        outp[b, (1 - h) * NSH:(2 - h) * NSH, js] = O[NSH:]
    if _trace:
        return outp, res
    return outp


# revision 44
# speedup vs baseline: 1.0230x; 1.0118x over previous
"""GatedAttention Trainium2 kernel.

Math (per batch b):
  Qw = x @ Wq + bq            (N, A)
  Kw = x @ Wk + bk            (N, A)
  g  = sigmoid(Qw @ Wv + bv)  (N,)
  S  = Qw @ Kw^T, diag -> -inf
  P  = softmax(S, axis=0)     (column softmax)
  out = (1-g)[:,None] * P + g[:,None] * I

Sharding: 8 cores = 4 batches x 2 column-halves of the score matrix.
Column softmax is independent per column, so no cross-core reduction.

Device layout: scores computed transposed, sT[j, i] tiles (j on partitions)
so the softmax reduction over i is a free-axis reduction. The i axis is
host-permuted so each core's diagonal block sits at i in [0, 2048) —
this keeps the program identical across cores (pure SPMD).

Pipeline per core:
  Projections (per 512-wide i-block, processed 4..7 then 0..3 in pairs):
  x arrives as [128, 1024] pair-tiles via SWDGE cast-DMA straight into
  fp32r (rounds in flight); weights are host-packed [128, 8, A] and land
  in two packed cast-DMAs each. The first block's Q accumulation is
  split into h0-3/h4-7 brackets fed by the sync and gpsimd DMA queues in
  parallel so the PE starts ~5us earlier and HAM never re-throttles.
  The gate (z = Qw @ Wv, g = 0.5+0.5*tanh(z/2); tanh shares the exp ACT
  table set) is software-pipelined one block behind, its PE ops
  interleaved between the next block's Q-groups so the ACT/DVE chain
  never stalls the strict-FIFO PE queue; the final gate lands inside
  the main loop's first tile. Diagonal planes dzp = ident * g are
  precomputed.
  Main loop per 128-column tile t: score matmuls -> PSUM, diag -1e30,
  exp (bf16 out, fp32 row-sum accum), 1/sum via fast reciprocal, bf16
  tensor_scalar (x 1/denom) + tensor_tensor (x (1-g_i)) passes, diag
  adds dzp, output streams out per 1024-wide chunk (bf16, host casts
  back to fp32).
"""
import numpy as np

import concourse.bacc as bacc
import concourse.mybir as mybir
import concourse.tile as tile
from concourse.bass_utils import run_bass_kernel_spmd

FP32 = mybir.dt.float32
FP32R = mybir.dt.float32r
BF16 = mybir.dt.bfloat16
AF = mybir.ActivationFunctionType
ALU = mybir.AluOpType

B, N, H, A = 4, 4096, 1024, 512
NSH = N // 2          # per-core column shard
NEG = -1.0e30

_CACHE = {}


def _build():
    nc = bacc.Bacc("TRN2", target_bir_lowering=False, debug=False, num_devices=8)
    xq = nc.dram_tensor("xq", [128, 8, N], FP32, kind="ExternalInput").ap()
    wq = nc.dram_tensor("wq", [H, A], FP32, kind="ExternalInput").ap()
    wk = nc.dram_tensor("wk", [H, A], FP32, kind="ExternalInput").ap()
    misc = nc.dram_tensor("misc", [128, 18], FP32, kind="ExternalInput").ap()
    out = nc.dram_tensor("out", [NSH, N], BF16, kind="ExternalOutput").ap()

    with tile.TileContext(nc) as tc:
        with (
            tc.tile_pool(name="const", bufs=1) as cpool,
            tc.tile_pool(name="proj_out", bufs=1) as qkpool,
            tc.tile_pool(name="gate", bufs=1) as gpool,
            tc.tile_pool(name="zrowps", bufs=1, space="PSUM") as zpool,
            tc.tile_pool(name="bcps", bufs=1, space="PSUM") as bps,
            tc.tile_pool(name="rowtmp", bufs=1) as rtmp,
        ):
            misc_sb = cpool.tile([128, 18], FP32, tag="misc", name="misc")
            nc.sync.dma_start(misc_sb[:], misc)

            # ---- PE warmup: dependency-free dummy matmuls issued while the
            # first operand DMAs are in flight, so the HAM activity window
            # opens before real work arrives (otherwise the first ~60 real
            # matmuls run at the cold 1.2 GHz rate). fp32 x fp32 on a
            # memset row — no fp32r rounding constraints involved.
            ones_f = cpool.tile([1, 128], FP32, tag="onesf", name="onesf")
            nc.vector.memset(ones_f[:], 1.0)
            warm = bps.tile([128, 512], FP32, tag="pb", name="warm")
            for _ in range(14):
                nc.tensor.matmul(warm[:, 0:128], ones_f[0:1, :], ones_f[0:1, :],
                                 start=True, stop=True)

            # ---- persistent projection outputs (fp32r) + gate planes ----
            qwt = [qkpool.tile([128, N], FP32R, tag=f"qwt{a}", name=f"qwt{a}") for a in range(4)]
            kwt = [qkpool.tile([128, NSH], FP32R, tag=f"kwt{a}", name=f"kwt{a}") for a in range(4)]
            g1m_bc = gpool.tile([128, N], BF16, tag="g1mbc", name="g1mbc")
            dzp = gpool.tile([128, NSH], BF16, tag="dzp", name="dzp")

            # gate stages, software-pipelined one i-block behind the
            # projections (stage A: z matmuls + tanh; B: (1-g) row +
            # broadcast; C: g row + diag plane)
            def gate_a(ib):
                sl = slice(ib * 512, (ib + 1) * 512)
                pzc = zpool.tile([2, 512], FP32, tag="zr", name="zr")
                for a in range(4):
                    nc.tensor.matmul(pzc[:], misc_r[:, 8 + 2 * a:10 + 2 * a],
                                     qwt[a][:, sl], start=(a == 0), stop=(a == 3))
                y_row = rtmp.tile([1, 512], FP32, tag="y", name="y")
                nc.scalar.activation(y_row[:], pzc[0:1, :], AF.Tanh,
                                     bias=misc_sb[0:1, 16:17], scale=0.5)
                return y_row

            def gate_b(ib, y_row):
                sl = slice(ib * 512, (ib + 1) * 512)
                g1m_row = rtmp.tile([1, 512], FP32R, tag="g1m", name="g1m")
                nc.vector.tensor_scalar(g1m_row[:], y_row[:],
                                        -0.5, 0.5, op0=ALU.mult, op1=ALU.add)
                pb = bps.tile([128, 512], FP32, tag="pb", name="pb")
                nc.tensor.matmul(pb[:], ones_r[0:1, :], g1m_row[:],
                                 start=True, stop=True)
                nc.vector.tensor_copy(g1m_bc[:, sl], pb[:])

            def gate_c(ib, y_row):
                if ib >= 4:
                    return
                sl = slice(ib * 512, (ib + 1) * 512)
                g_row = rtmp.tile([1, 512], FP32R, tag="g", name="g")
                nc.vector.tensor_scalar(g_row[:], y_row[:],
                                        0.5, 0.5, op0=ALU.mult, op1=ALU.add)
                pb2 = bps.tile([128, 512], FP32, tag="pb", name="pb")
                nc.tensor.matmul(pb2[:], ones_r[0:1, :], g_row[:],
                                 start=True, stop=True)
                nc.vector.tensor_tensor(dzp[:, sl], pb2[:], identrep[:],
                                        op=ALU.mult)

            # ---- projections (i-blocks 4..7 then 0..3, loaded in pairs) ----
            PROC = [4, 5, 6, 7, 0, 1, 2, 3]
            with (
                tc.tile_pool(name="wtiles", bufs=1) as wpool,
                tc.tile_pool(name="xslices", bufs=12) as xpool,
                tc.tile_pool(name="projps", bufs=6, space="PSUM") as ppool,
            ):
                # startup: packed weight DMAs + first x pair's h4-7 on the
                # gpsimd (SWDGE cast) queue, first x pair's h0-3 via sync
                # startup: small per-h weight DMAs interleaved with the
                # first block's x singles so the first Q group is fed at
                # the lowest latency; later blocks load x as 4 packed
                # [128, 2, 512] DMAs (host-repacked xq[128, 8, N]).
                lo0 = PROC[0] * 512
                wqr, wkr, xs0 = [], [], []
                for h in range(8):
                    wr = wpool.tile([128, A], FP32R, tag=f"wqr{h}", name=f"wqr{h}")
                    nc.gpsimd.dma_start(wr[:], wq[h * 128:(h + 1) * 128, :])
                    wqr.append(wr)
                    xt = xpool.tile([128, 2, 512], FP32R, tag="xr", name="xr")
                    nc.gpsimd.dma_start(xt[:, 0:1, :],
                                        xq[:, h:h + 1, lo0:lo0 + 512])
                    xs0.append(xt)

                # ---- constants (emitted after the startup DMAs so they
                # don't delay the first operand loads) ----
                io = cpool.tile([128, 128], mybir.dt.int32, tag="io", name="io")
                nc.gpsimd.iota(io[:], pattern=[[1, 128]], base=0,
                               channel_multiplier=-1)
                ident = cpool.tile([128, 128], FP32, tag="ident", name="ident")
                nc.vector.tensor_scalar(ident[:], io[:], 0, None, op0=ALU.is_equal)
                dneg = cpool.tile([128, 128], FP32, tag="dneg", name="dneg")
                nc.vector.tensor_scalar(dneg[:], ident[:], NEG, None, op0=ALU.mult)
                identrep = cpool.tile([128, 512], BF16, tag="idrep", name="idrep")
                for k in range(4):
                    nc.vector.tensor_copy(identrep[:, k * 128:(k + 1) * 128],
                                          ident[:])
                misc_r = cpool.tile([128, 18], FP32R, tag="miscr", name="miscr")
                nc.vector.tensor_copy(misc_r[:], misc_sb[:])
                ones_r = cpool.tile([1, 128], FP32R, tag="ones", name="ones")
                nc.vector.tensor_copy(ones_r[:], ones_f[:])

                prev = None
                y_prev = None
                for k, ib in enumerate(PROC):
                    sl = slice(ib * 512, (ib + 1) * 512)
                    if k == 0:
                        xs = [(xs0[h], 0) for h in range(8)]
                    else:
                        xs = []
                        for hp in range(4):
                            xt = xpool.tile([128, 2, 512], FP32R, tag="xr",
                                            name="xr")
                            nc.gpsimd.dma_start(
                                xt[:], xq[:, 2 * hp:2 * hp + 2, sl])
                            xs.append((xt, 0))
                            xs.append((xt, 1))
                    # deferred wk loads: not needed until the first K block
                    # (5th processed), so keep them off the gpsimd queue
                    # during the startup-critical x prefetch window
                    if k in (2, 3):
                        for h in range(4 * (k - 2), 4 * (k - 1)):
                            wr2 = wpool.tile([128, A], FP32R, tag=f"wkr{h}",
                                             name=f"wkr{h}")
                            nc.gpsimd.dma_start(wr2[:],
                                                wk[h * 128:(h + 1) * 128, :])
                            wkr.append(wr2)
                    for a in range(4):
                        pq = ppool.tile([128, 512], FP32, tag="ps", name="ps")
                        for h in range(8):
                            xt, m = xs[h]
                            nc.tensor.matmul(pq[:], wqr[h][:, a * 128:(a + 1) * 128],
                                             xt[:, m:m + 1, :],
                                             start=(h == 0), stop=(h == 7))
                        nc.scalar.activation(qwt[a][:, sl], pq[:],
                                             AF.Identity, bias=misc_sb[:, a:a + 1])
                        if prev is not None:
                            if a == 0:
                                y_prev = gate_a(prev)
                            elif a == 1:
                                gate_b(prev, y_prev)
                            elif a == 2:
                                gate_c(prev, y_prev)
                        if ib < 4:
                            pk = ppool.tile([128, 512], FP32, tag="ps", name="ps")
                            for h in range(8):
                                xt, m = xs[h]
                                nc.tensor.matmul(pk[:], wkr[h][:, a * 128:(a + 1) * 128],
                                                 xt[:, m:m + 1, :],
                                                 start=(h == 0), stop=(h == 7))
                            nc.scalar.activation(kwt[a][:, sl], pk[:],
                                                 AF.Identity, bias=misc_sb[:, 4 + a:5 + a])
                    prev = ib

            # ---- main loop over column tiles (output stays transposed).
            # The last pending gate (i-block 3) is emitted between t=0's
            # chunk groups for PE runway.
            with (
                tc.tile_pool(name="expp", bufs=4) as epool,
                tc.tile_pool(name="dsum", bufs=2) as dpool,
                tc.tile_pool(name="scoreps", bufs=3, space="PSUM") as sps,
            ):
                for t in range(16):
                    exp_t = epool.tile([128, N], BF16, tag="exp", name="exp")
                    dsum = dpool.tile([128, 5], FP32, tag="ds", name="ds")
                    dch = (t * 128) // 1024
                    for ch in range(4):
                        ps = sps.tile([128, 1024], FP32, tag="sc", name="sc")
                        for sub in range(2):
                            o = ch * 1024 + sub * 512
                            for a in range(4):
                                nc.tensor.matmul(ps[:, sub * 512:(sub + 1) * 512],
                                                 kwt[a][:, t * 128:(t + 1) * 128],
                                                 qwt[a][:, o:o + 512],
                                                 start=(a == 0), stop=(a == 3))
                        if ch == dch:
                            off = t * 128 - ch * 1024
                            nc.vector.tensor_add(ps[:, off:off + 128],
                                                 ps[:, off:off + 128], dneg[:])
                        if t == 15 and ch == 3:
                            # last chunk of the kernel: split the exp so the
                            # row sums (and thus the scale chain) start half
                            # a chunk earlier
                            nc.scalar.activation(exp_t[:, 3072:3584], ps[:, 0:512],
                                                 AF.Exp, accum_out=dsum[:, 3:4])
                            nc.scalar.activation(exp_t[:, 3584:4096], ps[:, 512:1024],
                                                 AF.Exp, accum_out=dsum[:, 4:5])
                        else:
                            nc.scalar.activation(exp_t[:, ch * 1024:(ch + 1) * 1024],
                                                 ps[:], AF.Exp,
                                                 accum_out=dsum[:, ch:ch + 1])
                        if t == 0 and prev is not None:
                            if ch == 0:
                                y_prev = gate_a(prev)
                            elif ch == 1:
                                gate_b(prev, y_prev)
                            elif ch == 2:
                                gate_c(prev, y_prev)
                                prev = None
                        # the x(1-g_i) factor is rcol-independent: apply it
                        # here, overlapped with the next chunk's matmuls, so
                        # only the fast x(1/denom) pass remains on the
                        # post-row-sum critical path
                        sl = slice(ch * 1024, (ch + 1) * 1024)
                        nc.vector.tensor_mul(exp_t[:, sl], exp_t[:, sl],
                                             g1m_bc[:, sl])
                    rsum = dpool.tile([128, 1], FP32, tag="r", name="r")
                    ncols = 5 if t == 15 else 4
                    nc.vector.tensor_reduce(rsum[:], dsum[:, 0:ncols],
                                            axis=mybir.AxisListType.X, op=ALU.add)
                    rcol = dpool.tile([128, 1], FP32, tag="r2", name="r2")
                    nc.vector.reciprocal_approx_fast(out=rcol[:], in_=rsum[:])
                    # last tile: diag chunk first so its extra dz add is off
                    # the final-DMA critical path
                    chorder = (dch, *(c for c in range(4) if c != dch)) if t == 15 \
                        else range(4)
                    for ch in chorder:
                        sl = slice(ch * 1024, (ch + 1) * 1024)
                        nc.vector.tensor_scalar(exp_t[:, sl], exp_t[:, sl],
                                                rcol[:], None, op0=ALU.mult)
                        if ch == dch:
                            nc.vector.tensor_add(exp_t[:, t * 128:(t + 1) * 128],
                                                 exp_t[:, t * 128:(t + 1) * 128],
                                                 dzp[:, t * 128:(t + 1) * 128])
                        nc.sync.dma_start(out[t * 128:(t + 1) * 128, sl],
                                          exp_t[:, sl])
    nc.compile()
    return nc


def kernel(x, Wq, bq, Wk, bk, Wv, bv, _trace=False):
    x = np.asarray(x, dtype=np.float32)
    if "nc" not in _CACHE:
        _CACHE["nc"] = _build()
    nc = _CACHE["nc"]

    misc = np.zeros((128, 18), dtype=np.float32)
    misc[:, 0:4] = np.asarray(bq, np.float32).reshape(4, 128).T
    misc[:, 4:8] = np.asarray(bk, np.float32).reshape(4, 128).T
    wv_c = np.asarray(Wv, np.float32).reshape(4, 128).T
    misc[:, 8:16:2] = wv_c
    misc[:, 9:16:2] = wv_c
    # tanh path evaluates tanh(0.5*z + bv/2)
    misc[:, 16] = 0.5 * np.float32(np.asarray(bv).reshape(())[()])
    wq_np = np.ascontiguousarray(np.asarray(Wq, np.float32))
    wk_np = np.ascontiguousarray(np.asarray(Wk, np.float32))

    in_maps = []
    for c in range(8):
        b, h = c // 2, c % 2
        xT = x[b].T  # (H, N)
        if h != 0:
            xT = np.concatenate([xT[:, NSH:], xT[:, :NSH]], axis=1)
        # packed [128, 8, N]: xp[p, h, i] = xT[h*128+p, i]
        xqc = np.ascontiguousarray(xT.reshape(8, 128, N).transpose(1, 0, 2))
        in_maps.append({"xq": xqc, "wq": wq_np, "wk": wk_np, "misc": misc})

    res = run_bass_kernel_spmd(nc, in_maps, list(range(8)), trace=_trace)

    outp = np.empty((B, N, N), dtype=np.float32)
    for c in range(8):
        b, h = c // 2, c % 2
        O = np.asarray(res.results[c]["out"], dtype=np.float32).T  # (N i_perm, NSH j)
        js = slice(h * NSH, (h + 1) * NSH)
        outp[b, h * NSH:(h + 1) * NSH, js] = O[:NSH]
        outp[b, (1 - h) * NSH:(2 - h) * NSH, js] = O[NSH:]
    if _trace:
        return outp, res
    return outp
